# revision 1
# baseline (speedup 1.0000x reference)
"""KIVI attention wrapper — Trainium2 Bass kernel, 8-way head-sharded.

Sharding: 16 heads / 8 cores = 2 heads per core (tensor parallel).
Per core: QKV^T in feature-major layout via PE-transposed X; KIVI 2-bit
fake-quant of K on device; scores computed transposed ([kpos, q]) so softmax
sum lands on a matmul ones-column; AllGather of per-core attention output;
c_proj column-sharded (each core produces 128 output columns, token-major
gathered on host).
"""
import sys
sys.path.insert(0, '/opt/trn_rl_repo')
import numpy as np

P = 128
TOK = 4096          # B*S = 4*1024
E = 1024
NB = 8              # embed 128-blocks
CH = 512            # token chunk
NCH = 8             # token 512-chunks
TB = 32             # token 128-blocks
MAGIC = 8388608.0   # 2^23: x + MAGIC - MAGIC == rint(x) for 0 <= x < 2^22
USE_F32R = True     # tf32 matmuls: 4x PE throughput, ~1e-3 component error

_CACHE = {}


def _build(sim_single=False):
    import concourse.bacc as bacc
    import concourse.mybir as mybir
    import concourse.tile as tile

    f32 = mybir.dt.float32
    fmm = mybir.dt.float32r if USE_F32R else mybir.dt.float32
    X = mybir.AxisListType.X
    ADD = mybir.AluOpType.add
    MULT = mybir.AluOpType.mult
    MAX = mybir.AluOpType.max
    SUB = mybir.AluOpType.subtract
    EXP = mybir.ActivationFunctionType.Exp

    nc = bacc.Bacc("TRN2", target_bir_lowering=False, debug=False,
                   num_devices=(1 if sim_single else 8))

    x_ap = nc.dram_tensor("x", [TOK, E], f32, kind="ExternalInput").ap()
    wqkv_ap = nc.dram_tensor("wqkv", [E, 384], f32, kind="ExternalInput").ap()
    bqkv_ap = nc.dram_tensor("bqkv", [P, 3], f32, kind="ExternalInput").ap()
    m8t_ap = nc.dram_tensor("m8t", [P, 32], f32, kind="ExternalInput").ap()
    wp_ap = nc.dram_tensor("wp", [E, P], f32, kind="ExternalInput").ap()
    bp_ap = nc.dram_tensor("bp", [P, 1], f32, kind="ExternalInput").ap()
    ident_ap = nc.dram_tensor("ident", [P, P], f32, kind="ExternalInput").ap()
    ones1_ap = nc.dram_tensor("ones1", [1, 64], f32, kind="ExternalInput").ap()
    yt_ap = nc.dram_tensor("yt", [P, TOK], f32, kind="ExternalOutput").ap()

    with tile.TileContext(nc) as tc:
        with tc.tile_pool(name="const", bufs=1) as constp, \
             tc.tile_pool(name="big", bufs=1) as bigp, \
             tc.tile_pool(name="dram", bufs=1, space="DRAM") as dramp:

            identt = constp.tile([P, P], f32)
            nc.sync.dma_start(identt[:], ident_ap)
            ones1t = constp.tile([1, 64], f32)
            nc.sync.dma_start(ones1t[:], ones1_ap)
            if USE_F32R:
                ones1r = constp.tile([1, 64], fmm, name="ones1r", tag="ones1r")
                nc.vector.tensor_copy(ones1r[:], ones1t[:])
            else:
                ones1r = ones1t
            m8tt = constp.tile([P, 32], f32)
            nc.sync.dma_start(m8tt[:], m8t_ap)
            bqkvt = constp.tile([P, 3], f32)
            nc.sync.dma_start(bqkvt[:], bqkv_ap)
            bpt = constp.tile([P, 1], f32)
            nc.sync.dma_start(bpt[:], bp_ap)
            onescol = constp.tile([P, 1], f32)
            nc.any.memset(onescol[:], 1.0)
            wts = []
            for eb in range(NB):
                wt = constp.tile([P, 384], f32, name=f"wt{eb}", tag=f"wt{eb}")
                nc.sync.dma_start(wt[:], wqkv_ap[eb * P:(eb + 1) * P, :])
                if USE_F32R:
                    wtr = constp.tile([P, 384], fmm, name=f"wtr{eb}",
                                      tag=f"wtr{eb}")
                    nc.vector.tensor_copy(wtr[:], wt[:])
                    wt = wtr
                wts.append(wt)
            wps = []
            for eb in range(NB):
                wpt = constp.tile([P, P], f32, name=f"wp{eb}", tag=f"wp{eb}")
                nc.sync.dma_start(wpt[:], wp_ap[eb * P:(eb + 1) * P, :])
                if USE_F32R:
                    wpr = constp.tile([P, P], fmm, name=f"wpr{eb}",
                                      tag=f"wpr{eb}")
                    nc.vector.tensor_copy(wpr[:], wpt[:])
                    wpt = wpr
                wps.append(wpt)

            # persistent feature-major tensors [128 = 2 heads x 64, 4096 tok]
            qT = bigp.tile([P, TOK], fmm, tag="qT")
            kT = bigp.tile([P, TOK], f32, tag="kT")
            vT = bigp.tile([P, TOK], f32, tag="vT")
            kdT = bigp.tile([P, TOK], fmm, tag="kdT")
            oT = bigp.tile([P, TOK], f32, tag="oT")
            qkvT = [qT, kT, vT]

            # ---------------- Stage 1: X^T + QKV^T GEMM ----------------
            with tc.tile_pool(name="s1", bufs=2) as s1p, \
                 tc.tile_pool(name="s1ps", bufs=5, space="PSUM") as s1ps, \
                 tc.tile_pool(name="g1ps", bufs=3, space="PSUM") as g1ps:
                for ch in range(NCH):
                    xns = []
                    for tb in range(4):
                        xn = s1p.tile([P, E], f32, name=f"xn{tb}", tag=f"xn{tb}")
                        nc.sync.dma_start(
                            xn[:], x_ap[ch * CH + tb * P: ch * CH + (tb + 1) * P, :])
                        xns.append(xn)
                    xTs = []
                    for eb in range(NB):
                        xT = s1p.tile([P, CH], fmm, name=f"xT{eb}", tag=f"xT{eb}")
                        xTs.append(xT)
                    for eb in range(NB):
                        for tb in range(4):
                            pst = s1ps.tile([P, P], f32, tag="pst")
                            nc.tensor.transpose(
                                pst[:], xns[tb][:, eb * P:(eb + 1) * P], identt[:])
                            dst = xTs[eb][:, tb * P:(tb + 1) * P]
                            if (eb + tb) % 2 == 0:
                                nc.vector.tensor_copy(dst, pst[:])
                            else:
                                nc.scalar.copy(dst, pst[:])
                    for m in range(3):
                        gps = g1ps.tile([P, CH], f32, tag="gps")
                        for eb in range(NB):
                            nc.tensor.matmul(
                                gps[:], wts[eb][:, m * P:(m + 1) * P], xTs[eb][:],
                                start=(eb == 0), stop=(eb == NB - 1))
                        nc.vector.tensor_tensor(
                            qkvT[m][:, ch * CH:(ch + 1) * CH], gps[:],
                            bqkvt[:, m:m + 1].to_broadcast((P, CH)), ADD)

            # ---------------- Stage 2: KIVI fake-quant of K -------------
            # ---------------- Stage 3: V transpose (+ones col) ----------
            vt_tiles = []
            with tc.tile_pool(name="s2", bufs=2) as s2p, \
                 tc.tile_pool(name="s2ps", bufs=2, space="PSUM") as s2ps:
                for kb in range(TB):
                    ps_a = s2ps.tile([P, P], f32, tag="ps_a")
                    nc.tensor.transpose(ps_a[:], kT[:, kb * P:(kb + 1) * P], identt[:])
                    knat = s2p.tile([P, P], f32, tag="knat")
                    nc.scalar.copy(knat[:], ps_a[:])
                    gmax = s2p.tile([P, 32], f32, tag="gmax")
                    nc.vector.tensor_reduce(
                        gmax[:], knat[:].rearrange("p (g f) -> p g f", f=4),
                        axis=X, op=MAX, apply_absolute_value=True)
                    scalet = s2p.tile([P, 32], f32, tag="scalet")
                    nc.vector.tensor_scalar_mul(scalet[:], gmax[:], 1.0 / 1.5)
                    rs = s2p.tile([P, 32], f32, tag="rs")
                    nc.vector.reciprocal(rs[:], scalet[:])
                    kd = s2p.tile([P, P], f32, tag="kd")
                    kd_g = kd[:].rearrange("p (g f) -> p g f", f=4)
                    knat_g = knat[:].rearrange("p (g f) -> p g f", f=4)
                    nc.vector.tensor_tensor(
                        kd_g, knat_g, rs[:, :, None].to_broadcast((P, 32, 4)), MULT)
                    nc.vector.tensor_scalar(kd[:], kd[:], 1.5, MAGIC,
                                            ADD, ADD)
                    nc.vector.tensor_scalar(kd[:], kd[:], MAGIC, 1.5,
                                            SUB, SUB)
                    nc.vector.tensor_tensor(
                        kd_g, kd_g, scalet[:, :, None].to_broadcast((P, 32, 4)), MULT)
                    ps_b = s2ps.tile([P, P], f32, tag="ps_b")
                    nc.tensor.transpose(ps_b[:], kd[:], identt[:])
                    nc.scalar.copy(kdT[:, kb * P:(kb + 1) * P], ps_b[:])

                    # V natural tiles, one per head, with ones column at 64
                    ps_v = s2ps.tile([P, P], f32, tag="ps_v")
                    nc.tensor.transpose(ps_v[:], vT[:, kb * P:(kb + 1) * P], identt[:])
                    vh = []
                    for h in range(2):
                        v = bigp.tile([P, 65], fmm, name=f"v{kb}_{h}",
                                      tag=f"v{kb}_{h}")
                        nc.vector.tensor_copy(v[:, 64:65], onescol[:])
                        nc.scalar.copy(
                            v[:, 0:64], ps_v[:, h * 64:(h + 1) * 64])
                        vh.append(v)
                    vt_tiles.append(vh)

            # ---------------- Stage 4: attention ------------------------
            with tc.tile_pool(name="s4", bufs=2) as s4p, \
                 tc.tile_pool(name="s4ps", bufs=3, space="PSUM") as s4ps, \
                 tc.tile_pool(name="avps", bufs=2, space="PSUM") as avps, \
                 tc.tile_pool(name="rps", bufs=2, space="PSUM") as rps:
                for b in range(4):
                    for h in range(2):
                        hs = slice(h * 64, (h + 1) * 64)
                        for qc in range(2):
                            q0 = b * 1024 + qc * CH
                            es = []
                            for kb in range(8):
                                gkb = b * 8 + kb
                                ps_s = s4ps.tile([P, CH], f32, tag="ps_s")
                                nc.tensor.matmul(
                                    ps_s[:],
                                    kdT[hs, gkb * P:(gkb + 1) * P],
                                    qT[hs, q0:q0 + CH],
                                    start=True, stop=True)
                                e = s4p.tile([P, CH], fmm, name=f"e{kb}",
                                             tag=f"e{kb}")
                                nc.scalar.activation(
                                    e[:], ps_s[:], EXP,
                                    bias=m8tt[:, gkb:gkb + 1], scale=0.125)
                                es.append(e)
                            ps_av = avps.tile([65, CH], f32, tag="ps_av")
                            for kb in range(8):
                                nc.tensor.matmul(
                                    ps_av[:], vt_tiles[b * 8 + kb][h][:], es[kb][:],
                                    start=(kb == 0), stop=(kb == 7))
                            rS = s4p.tile([1, CH], fmm, tag="rS")
                            with nc.allow_low_precision(reason="tf32 recip"):
                                nc.vector.reciprocal(rS[:], ps_av[64:65, :])
                            ps_r = rps.tile([64, CH], f32, tag="ps_r")
                            nc.tensor.matmul(ps_r[:], ones1r[:], rS[:],
                                             start=True, stop=True)
                            rrep = s4p.tile([64, CH], f32, tag="rrep")
                            nc.scalar.copy(rrep[:], ps_r[:])
                            nc.vector.tensor_tensor(
                                oT[hs, q0:q0 + CH], ps_av[0:64, :], rrep[:], MULT)

            # ---------------- Stage 5: AllGather + c_proj ----------------
            agin = dramp.tile([P, TOK], f32, tag="agin")
            agout = dramp.tile([8, P, TOK], f32, tag="agout",
                               addr_space=("Local" if sim_single else "Shared"))
            nc.gpsimd.dma_start(agin[:], oT[:])
            if sim_single:
                for r in range(8):
                    nc.gpsimd.dma_start(agout[r], agin[:])
            else:
                nc.gpsimd.collective_compute(
                    "AllGather", mybir.AluOpType.bypass,
                    replica_groups=[list(range(8))],
                    ins=[agin[:]], outs=[agout[:]])
            with tc.tile_pool(name="s5", bufs=3) as s5p, \
                 tc.tile_pool(name="s5ps", bufs=2, space="PSUM") as s5ps:
                for nch in range(NCH):
                    ps_p = s5ps.tile([P, CH], f32, tag="ps_p")
                    for kb2 in range(NB):
                        rt = s5p.tile([P, CH], f32, tag="rt")
                        nc.gpsimd.dma_start(
                            rt[:], agout[kb2, :, nch * CH:(nch + 1) * CH])
                        if USE_F32R:
                            rtr = s5p.tile([P, CH], fmm, tag="rtr")
                            if kb2 % 2 == 0:
                                nc.vector.tensor_copy(rtr[:], rt[:])
                            else:
                                nc.scalar.copy(rtr[:], rt[:])
                            rt = rtr
                        nc.tensor.matmul(ps_p[:], wps[kb2][:], rt[:],
                                         start=(kb2 == 0), stop=(kb2 == NB - 1))
                    yts = s5p.tile([P, CH], f32, tag="yts")
                    nc.vector.tensor_tensor(
                        yts[:], ps_p[:], bpt[:].to_broadcast((P, CH)), ADD)
                    nc.sync.dma_start(yt_ap[:, nch * CH:(nch + 1) * CH], yts[:])

    nc.compile()
    return nc


def make_in_maps(hidden_states, attention_mask, w_attn, b_attn, w_proj, b_proj):
    x = np.ascontiguousarray(np.asarray(hidden_states, np.float32).reshape(TOK, E))
    mask = np.asarray(attention_mask, np.float32)
    wa = np.asarray(w_attn, np.float32)
    ba = np.asarray(b_attn, np.float32)
    wpf = np.asarray(w_proj, np.float32)
    bp = np.asarray(b_proj, np.float32)

    m8 = (mask * np.float32(0.125)).reshape(4, 8, 128)
    m8t = np.ascontiguousarray(m8.transpose(2, 0, 1).reshape(128, 32))
    ident = np.eye(P, dtype=np.float32)
    ones1 = np.ones((1, 64), dtype=np.float32)

    in_maps = []
    for c in range(8):
        cs = slice(c * P, (c + 1) * P)
        wqkv = np.ascontiguousarray(np.concatenate(
            [wa[:, cs], wa[:, 1024 + c * P:1024 + (c + 1) * P],
             wa[:, 2048 + c * P:2048 + (c + 1) * P]], axis=1))
        bqkv = np.ascontiguousarray(np.stack(
            [ba[cs], ba[1024 + c * P:1024 + (c + 1) * P],
             ba[2048 + c * P:2048 + (c + 1) * P]], axis=1))
        in_maps.append({
            "x": x, "wqkv": wqkv, "bqkv": bqkv, "m8t": m8t,
            "wp": np.ascontiguousarray(wpf[:, cs]),
            "bp": np.ascontiguousarray(bp[cs][:, None]),
            "ident": ident, "ones1": ones1,
        })
    return in_maps


def kernel(hidden_states, attention_mask, w_attn, b_attn, w_proj, b_proj):
    from concourse import bass_utils
    if "nc" not in _CACHE:
        _CACHE["nc"] = _build()
    nc = _CACHE["nc"]
    in_maps = make_in_maps(hidden_states, attention_mask, w_attn, b_attn,
                           w_proj, b_proj)
    res = bass_utils.run_bass_kernel_spmd(nc, in_maps, core_ids=list(range(8)))
    y = np.empty((TOK, E), dtype=np.float32)
    for c in range(8):
        y[:, c * P:(c + 1) * P] = res.results[c]["yt"].T
    return y.reshape(4, 1024, E)



# revision 13
# speedup vs baseline: 182.6301x; 182.6301x over previous
"""KIVI attention wrapper — Trainium2 Bass kernel, 8-way head-sharded.

Sharding: 16 heads / 8 cores = 2 heads per core (tensor parallel) for
QKV + attention; token-parallel (512 tokens per core) for c_proj after an
AllToAll of the per-head attention output.

Per core:
- X^T is provided pre-transposed by the host; QKV^T = W^T X^T directly in
  feature-major layout (f32r / tf32 matmuls).
- KIVI 2-bit fake-quant of K done entirely on DVE via 32x32 blockwise
  stream-transposes (group-of-4 absmax along the free axis).
- Scores computed transposed ([kpos, q]); the two heads' score matmuls are
  row-packed (K=64 at array rows 0-63 / 64-127) and run concurrently.
- exp batched over [128, 2048] PSUM regions (2 kb-blocks x 2 heads).
- The additive attention mask is folded into the V tiles (and the softmax
  denominator column) as exp(mask/8) row scaling — exact.
- Softmax normalization deferred: unnormalized O~ and the per-(head,q)
  denominators travel through the AllToAll; one [16, 512] reciprocal per
  core afterwards.
- c_proj token-sharded: each core computes all 1024 output columns for its
  512 tokens; output returned column-major [1024, 512] per core.
"""
import sys
sys.path.insert(0, '/opt/trn_rl_repo')
import numpy as np

P = 128
TOK = 4096          # B*S = 4*1024
E = 1024
NB = 8              # embed 128-blocks
CH = 512            # token chunk
NCH = 8             # token 512-chunks
MAGIC = 8388608.0   # 2^23: x + MAGIC - MAGIC == rint(x) for |x| < 2^22
OSH = P * CH + 2 * CH   # flat a2a shard: o block + 2 denominator rows

_CACHE = {}


def _build(sim_single=False):
    import concourse.bacc as bacc
    import concourse.mybir as mybir
    import concourse.tile as tile

    f32 = mybir.dt.float32
    fmm = mybir.dt.float32r
    X = mybir.AxisListType.X
    ADD = mybir.AluOpType.add
    MULT = mybir.AluOpType.mult
    MAX = mybir.AluOpType.max
    SUB = mybir.AluOpType.subtract
    EXP = mybir.ActivationFunctionType.Exp
    COPY = mybir.ActivationFunctionType.Copy

    nc = bacc.Bacc("TRN2", target_bir_lowering=False, debug=False,
                   num_devices=(1 if sim_single else 8))

    xt_ap = nc.dram_tensor("xt", [E, TOK], fmm, kind="ExternalInput").ap()
    wqkv_ap = nc.dram_tensor("wqkv", [E, 384], fmm, kind="ExternalInput").ap()
    bqkv_ap = nc.dram_tensor("bqkv", [P, 3], f32, kind="ExternalInput").ap()
    m8t_ap = nc.dram_tensor("m8t", [P, 32], f32, kind="ExternalInput").ap()
    wp_ap = nc.dram_tensor("wp", [E, E], fmm, kind="ExternalInput").ap()
    bpt_ap = nc.dram_tensor("bpt", [P, 8], f32, kind="ExternalInput").ap()
    e16_ap = nc.dram_tensor("e16", [16, E], fmm, kind="ExternalInput").ap()
    ident_ap = nc.dram_tensor("ident", [P, P], f32, kind="ExternalInput").ap()
    yt_ap = nc.dram_tensor("yt", [E, CH], f32, kind="ExternalOutput").ap()

    with tile.TileContext(nc) as tc:
        with tc.tile_pool(name="const", bufs=1) as constp, \
             tc.tile_pool(name="big", bufs=1) as bigp, \
             tc.tile_pool(name="xbp", bufs=12) as xbp, \
             tc.tile_pool(name="s1w", bufs=2) as s1w, \
             tc.tile_pool(name="qw", bufs=2) as qw, \
             tc.tile_pool(name="esp", bufs=2) as esp, \
             tc.tile_pool(name="otp", bufs=2) as otp, \
             tc.tile_pool(name="s5o", bufs=1) as s5o, \
             tc.tile_pool(name="s5p", bufs=2) as s5p, \
             tc.tile_pool(name="pA", bufs=2, space="PSUM") as pA, \
             tc.tile_pool(name="pB", bufs=1, space="PSUM") as pB, \
             tc.tile_pool(name="pC", bufs=1, space="PSUM") as pC, \
             tc.tile_pool(name="dram", bufs=1, space="DRAM") as dramp:

            identt = constp.tile([P, P], f32)
            nc.sync.dma_start(identt[:], ident_ap)
            bqkvt = constp.tile([P, 3], f32)
            nc.sync.dma_start(bqkvt[:], bqkv_ap)
            bptt = constp.tile([P, 8], f32)
            nc.sync.dma_start(bptt[:], bpt_ap)
            e16t = constp.tile([16, E], fmm)
            nc.sync.dma_start(e16t[:], e16_ap)
            m8tt = constp.tile([P, 32], f32)
            nc.sync.dma_start(m8tt[:], m8t_ap)
            emask = constp.tile([P, 32], fmm, name="emask", tag="emask")
            nc.scalar.activation(emask[:], m8tt[:], EXP)
            wts = []
            for eb in range(NB):
                wt = constp.tile([P, 384], fmm, name=f"wt{eb}", tag=f"wt{eb}")
                nc.sync.dma_start(wt[:], wqkv_ap[eb * P:(eb + 1) * P, :])
                wts.append(wt)
            wps = []
            for eb in range(NB):
                wpt = constp.tile([P, E], fmm, name=f"wp{eb}", tag=f"wp{eb}")
                nc.sync.dma_start(wpt[:], wp_ap[eb * P:(eb + 1) * P, :])
                wps.append(wpt)

            # persistent feature-major tensors [128 = 2 heads x 64, 4096 tok]
            qT = bigp.tile([P, TOK], fmm, tag="qT")
            kdT = bigp.tile([P, TOK], fmm, tag="kdT")
            vtiles = []   # [gkb][h] -> [128 kpos, 65] (64 feats + emask col)

            agin = dramp.tile([NCH, OSH], fmm, tag="agin")
            agout = dramp.tile([NCH, OSH], fmm, tag="agout")

            for ch in range(NCH):
                b = ch // 2
                t0 = ch * CH
                # ---------------- S1: QKV GEMM (X^T from host) -----------
                xbs = []
                for eb in range(NB):
                    xb = xbp.tile([P, CH], fmm, name=f"xb{ch}_{eb}", tag="xb")
                    nc.sync.dma_start(
                        xb[:], xt_ap[eb * P:(eb + 1) * P, t0:t0 + CH])
                    xbs.append(xb)
                kc = s1w.tile([P, CH], f32, tag="kc")
                vc = s1w.tile([P, CH], f32, tag="vc")
                dsts = [qT[:, t0:t0 + CH], kc[:], vc[:]]
                for m in range(3):
                    gps = pA.tile([P, CH], f32, tag="pa")
                    for eb in range(NB):
                        nc.tensor.matmul(
                            gps[:], wts[eb][:, m * P:(m + 1) * P], xbs[eb][:],
                            start=(eb == 0), stop=(eb == NB - 1))
                    nc.vector.tensor_tensor(
                        dsts[m], gps[:],
                        bqkvt[:, m:m + 1].to_broadcast((P, CH)), ADD)

                # ---------------- S2a: KIVI fake-quant of K (DVE) --------
                kq = qw.tile([P, CH], f32, tag="kq")
                nc.vector.transpose(kq[:], kc[:])
                gmax = qw.tile([P, P], f32, tag="gmax")
                nc.vector.tensor_reduce(
                    gmax[:], kq[:].rearrange("p (g f) -> p g f", f=4),
                    axis=X, op=MAX, apply_absolute_value=True)
                rs = qw.tile([P, P], f32, tag="rs")
                nc.vector.reciprocal(rs[:], gmax[:])
                kd = qw.tile([P, CH], f32, tag="kd")
                kd_g = kd[:].rearrange("p (g f) -> p g f", f=4)
                kq_g = kq[:].rearrange("p (g f) -> p g f", f=4)
                nc.gpsimd.tensor_tensor(
                    kd_g, kq_g, rs[:, :, None].to_broadcast((P, P, 4)), MULT)
                # codes = rint(1.5*kd + 1.5) in [0,3]: affine, then magic
                # rounding (z + 2^23 - 2^23 == rint(z) for 0 <= z < 2^22)
                nc.vector.tensor_scalar(kd[:], kd[:], 1.5, 1.5, MULT, ADD)
                nc.vector.tensor_scalar(kd[:], kd[:], MAGIC, MAGIC, ADD, SUB)
                nc.gpsimd.tensor_scalar_sub(kd[:], kd[:], 1.5)
                # dequant*1.5: (codes-1.5)*absmax  (the 1/1.5 is folded into
                # W_q on the host)
                kdq = qw.tile([P, CH], f32, tag="kdq")
                nc.gpsimd.tensor_tensor(
                    kdq[:].rearrange("p (g f) -> p g f", f=4), kd_g,
                    gmax[:, :, None].to_broadcast((P, P, 4)), MULT)
                kdn = qw.tile([P, CH], f32, tag="kdn")
                nc.vector.transpose(kdn[:], kdq[:])
                nc.gpsimd.tensor_copy(kdT[:, t0:t0 + CH], kdn[:])

                # ---------------- S2b: V tiles (natural, mask-scaled) ----
                for tb in range(4):
                    gkb = ch * 4 + tb
                    ps_v = pA.tile([P, P], f32, tag="pa")
                    nc.tensor.transpose(
                        ps_v[:], vc[:, tb * P:(tb + 1) * P], identt[:])
                    vh = []
                    for h in range(2):
                        v = bigp.tile([P, 65], fmm, name=f"v{gkb}_{h}",
                                      tag=f"v{gkb}_{h}")
                        nc.vector.tensor_tensor(
                            v[:, 0:64], ps_v[:, h * 64:(h + 1) * 64],
                            emask[:, gkb:gkb + 1].to_broadcast((P, 64)), MULT)
                        nc.vector.tensor_copy(
                            v[:, 64:65], emask[:, gkb:gkb + 1])
                        vh.append(v)
                    vtiles.append(vh)

                # ---------------- S3: attention for batch b --------------
                if ch % 2 == 1:
                    for qc in range(2):
                        j = b * 2 + qc
                        q0 = j * CH
                        cav = [pC.tile([65, CH], f32, name=f"cav{j}_{h}",
                                       tag=f"cav{h}")
                               for h in range(2)]
                        for pair in range(4):
                            ps_s = pB.tile([P, 4 * CH], f32, tag="pb")
                            for kb2 in range(2):
                                gkb = b * 8 + pair * 2 + kb2
                                for h in range(2):
                                    sl = (kb2 * 2 + h) * CH
                                    nc.tensor.matmul(
                                        ps_s[:, sl:sl + CH],
                                        kdT[h * 64:(h + 1) * 64,
                                            gkb * P:(gkb + 1) * P],
                                        qT[h * 64:(h + 1) * 64, q0:q0 + CH],
                                        start=True, stop=True)
                            es = esp.tile([P, 4 * CH], fmm, tag="es")
                            nc.scalar.activation(es[:], ps_s[:], EXP,
                                                 scale=0.125)
                            for kb2 in range(2):
                                gkb = b * 8 + pair * 2 + kb2
                                for h in range(2):
                                    sl = (kb2 * 2 + h) * CH
                                    nc.tensor.matmul(
                                        cav[h][:], vtiles[gkb][h][:],
                                        es[:, sl:sl + CH],
                                        start=(pair == 0 and kb2 == 0),
                                        stop=(pair == 3 and kb2 == 1),
                                        skip_group_check=True)
                        oTj = otp.tile([P, CH], fmm, tag="oTj")
                        dens = [otp.tile([1, CH], fmm, name=f"den{j}_{h}",
                                         tag=f"den{h}") for h in range(2)]
                        for h in range(2):
                            nc.vector.tensor_copy(
                                oTj[h * 64:(h + 1) * 64, :], cav[h][0:64, :])
                            nc.vector.tensor_copy(
                                dens[h][:], cav[h][64:65, :])
                        nc.sync.dma_start(
                            agin[j, 0:P * CH].rearrange("(p f) -> p f", p=P),
                            oTj[:])
                        for h in range(2):
                            nc.sync.dma_start(
                                agin[j, P * CH + h * CH:P * CH + (h + 1) * CH]
                                .rearrange("(p f) -> p f", p=1),
                                dens[h][:])

            # ---------------- S4: AllToAll ---------------------------
            if sim_single:
                nc.gpsimd.dma_start(agout[:], agin[:])
            else:
                nc.gpsimd.collective_compute(
                    "AllToAll", mybir.AluOpType.bypass,
                    replica_groups=[list(range(8))],
                    ins=[agin[:]], outs=[agout[:]])

            # ---------------- S5: normalize + c_proj (my 512 tokens) --
            rcpin = constp.tile([16, CH], fmm, tag="rcpin")
            nc.sync.dma_start(rcpin[0:8, :], agout[:, P * CH:P * CH + CH])
            nc.sync.dma_start(rcpin[8:16, :], agout[:, P * CH + CH:])
            rcpt = constp.tile([16, CH], fmm, tag="rcpt")
            with nc.allow_low_precision(reason="tf32 recip"):
                nc.vector.reciprocal(rcpt[:], rcpin[:])
            ogs = []
            for fb in range(NB):
                og = s5o.tile([P, CH], fmm, name=f"og{fb}", tag=f"og{fb}")
                nc.sync.dma_start(
                    og[:], agout[fb, 0:P * CH].rearrange("(p f) -> p f", p=P))
                ps_r = pA.tile([P, CH], f32, tag="pa")
                nc.tensor.matmul(ps_r[:], e16t[:, fb * P:(fb + 1) * P],
                                 rcpt[:], start=True, stop=True)
                rrep = s5p.tile([P, CH], f32, tag="rrep")
                nc.scalar.copy(rrep[:], ps_r[:])
                nc.vector.tensor_tensor(og[:], og[:], rrep[:], MULT)
                ogs.append(og)
            for ob in range(NB):
                ps_p = pA.tile([P, CH], f32, tag="pa")
                for fb in range(NB):
                    nc.tensor.matmul(
                        ps_p[:], wps[fb][:, ob * P:(ob + 1) * P], ogs[fb][:],
                        start=(fb == 0), stop=(fb == NB - 1))
                yts = s5p.tile([P, CH], f32, tag="yts")
                nc.vector.tensor_tensor(
                    yts[:], ps_p[:],
                    bptt[:, ob:ob + 1].to_broadcast((P, CH)), ADD)
                nc.sync.dma_start(yt_ap[ob * P:(ob + 1) * P, :], yts[:])

    nc.compile()
    return nc


def make_in_maps(hidden_states, attention_mask, w_attn, b_attn, w_proj, b_proj):
    x = np.asarray(hidden_states, np.float32).reshape(TOK, E)
    xt = np.ascontiguousarray(x.T)
    mask = np.asarray(attention_mask, np.float32)
    wa = np.asarray(w_attn, np.float32)
    ba = np.asarray(b_attn, np.float32)
    wpf = np.ascontiguousarray(np.asarray(w_proj, np.float32))
    bp = np.asarray(b_proj, np.float32)

    m8 = (mask * np.float32(0.125)).reshape(4, 8, 128)
    m8t = np.ascontiguousarray(m8.transpose(2, 0, 1).reshape(128, 32))
    ident = np.eye(P, dtype=np.float32)
    bpt = np.ascontiguousarray(bp.reshape(8, P).T)
    e16 = np.zeros((16, E), dtype=np.float32)
    for r in range(16):
        h = 2 * r if r < 8 else 2 * (r - 8) + 1
        fb, half = h // 2, h % 2
        e16[r, fb * P + half * 64: fb * P + half * 64 + 64] = 1.0

    in_maps = []
    for c in range(8):
        cs = slice(c * P, (c + 1) * P)
        wqkv = np.concatenate(
            [wa[:, cs] * np.float32(2.0 / 3.0),
             wa[:, 1024 + c * P:1024 + (c + 1) * P],
             wa[:, 2048 + c * P:2048 + (c + 1) * P]], axis=1)
        bqkv = np.stack(
            [ba[cs] * np.float32(2.0 / 3.0),
             ba[1024 + c * P:1024 + (c + 1) * P],
             ba[2048 + c * P:2048 + (c + 1) * P]], axis=1)
        in_maps.append({
            "xt": xt, "wqkv": np.ascontiguousarray(wqkv),
            "bqkv": np.ascontiguousarray(bqkv), "m8t": m8t,
            "wp": wpf, "bpt": bpt, "e16": e16, "ident": ident,
        })
    return in_maps


def kernel(hidden_states, attention_mask, w_attn, b_attn, w_proj, b_proj):
    from concourse import bass_utils
    if "nc" not in _CACHE:
        _CACHE["nc"] = _build()
    nc = _CACHE["nc"]
    in_maps = make_in_maps(hidden_states, attention_mask, w_attn, b_attn,
                           w_proj, b_proj)
    res = bass_utils.run_bass_kernel_spmd(nc, in_maps, core_ids=list(range(8)))
    y = np.empty((TOK, E), dtype=np.float32)
    for c in range(8):
        y[c * CH:(c + 1) * CH, :] = res.results[c]["yt"].T
    return y.reshape(4, 1024, E)
